# revision 1
# baseline (speedup 1.0000x reference)
"""GRU kernel for Trainium2, 8 NeuronCores, data-parallel over batch.

Reference computation (per sample b, step t):
  r = sigmoid(x_ir[t] + h @ W_hr.T + b_hr)        x_ir = X @ W_ir.T + b_ir
  z = sigmoid(x_iz[t] + h @ W_hz.T + b_hz)
  n = tanh  (x_in[t] + r * (h @ W_hn.T + b_hn))
  h = (1-z)*n + z*h

Sharding: batch 64 -> 8 per core; weights replicated; recurrence core-local.
Dataflow is fully transposed on-chip: hidden state lives as hT packed
[128 partitions, (hc=8, b=8)] so gate matmuls (W stationary via LDWEIGHTS,
hT moving) produce [H_chunk, batch] tiles directly and elementwise ops run
on dense [128, 64] tiles.  Matmul operands are bf16 (FWL-eligible weights);
the master h state, x-gates and all elementwise math stay fp32.
"""

import numpy as np
import ml_dtypes

import concourse.bass as bass
import concourse.mybir as mybir
import concourse.tile as tile
from concourse import bacc
from concourse.bass import ds
from concourse.bass_utils import run_bass_kernel_spmd

B, S, I, H = 64, 1024, 1024, 1024
NCORES = 8
BL = B // NCORES          # 8 sequences per core
NCH = 3 * H // 128        # 24 output row-chunks (gates stacked r,z,n)
KCH = H // 128            # 8 contraction chunks
MT = 16                   # phase-1 moving tiles of 512 cols over S*BL=8192
MTW = (S * BL) // MT      # 512

BF16 = mybir.dt.bfloat16
F32 = mybir.dt.float32
AF = mybir.ActivationFunctionType
PE = mybir.EngineType.PE

_built = None


def _build():
    nc = bacc.Bacc("TRN2", target_bir_lowering=False, debug=False,
                   num_devices=NCORES)

    xt = nc.dram_tensor("xt", [I, S * BL], BF16, kind="ExternalInput")
    whi = nc.dram_tensor("whi", [128, NCH * KCH * 128], BF16, kind="ExternalInput")
    whh = nc.dram_tensor("whh", [128, NCH * KCH * 128], BF16, kind="ExternalInput")
    xbias = nc.dram_tensor("xbias", [128, NCH], F32, kind="ExternalInput")
    bhn = nc.dram_tensor("bhn", [128, KCH * BL], F32, kind="ExternalInput")
    h0 = nc.dram_tensor("h0", [128, KCH * BL], F32, kind="ExternalInput")
    xg = nc.dram_tensor("xg", [3 * H, S * BL], F32)  # internal scratch
    outT = nc.dram_tensor("outT", [S, H, BL], F32, kind="ExternalOutput")

    xt_v = xt.ap().rearrange("(kc p) m -> p kc m", p=128)          # [128,8,8192]
    xg_w = xg.ap().rearrange("(n p) m -> p n m", p=128)            # [128,24,8192]
    xg_r = xg.ap().rearrange("(n p) (s b) -> p n s b", p=128, b=BL)  # [128,24,1024,8]
    outT_v = outT.ap().rearrange("s (hc p) b -> p s hc b", p=128)  # [128,1024,8,8]

    with tile.TileContext(nc) as tc:
        with tc.tile_pool(name="const", bufs=1) as cpool:
            whi_sb = cpool.tile([128, NCH * KCH * 128], BF16)
            whh_sb = cpool.tile([128, NCH * KCH * 128], BF16)
            xbias_sb = cpool.tile([128, NCH], F32)
            bhn_sb = cpool.tile([128, KCH * BL], F32)
            hf = cpool.tile([128, KCH * BL], F32)    # fp32 master hidden state
            hb = cpool.tile([128, KCH * BL], BF16)   # bf16 matmul operand copy
            nc.sync.dma_start(out=whi_sb[:], in_=whi.ap())
            nc.sync.dma_start(out=whh_sb[:], in_=whh.ap())
            nc.sync.dma_start(out=xbias_sb[:], in_=xbias.ap())
            nc.sync.dma_start(out=bhn_sb[:], in_=bhn.ap())
            nc.sync.dma_start(out=hf[:], in_=h0.ap())
            nc.vector.tensor_copy(hb[:], hf[:])

            # ---------------- Phase 1: x-gates = Wi_stack @ X^T + bias ----
            with tc.tile_pool(name="p1x", bufs=3) as xpool, \
                 tc.tile_pool(name="p1g", bufs=4) as gpool, \
                 tc.tile_pool(name="p1ps", bufs=3, space="PSUM") as ps1:
                for mt in range(MT):
                    xt_sb = xpool.tile([128, KCH, MTW], BF16)
                    nc.sync.dma_start(out=xt_sb[:],
                                      in_=xt_v[:, :, mt * MTW:(mt + 1) * MTW])
                    for n_ in range(NCH):
                        ps = ps1.tile([128, MTW], F32)
                        for kc in range(KCH):
                            w0 = (n_ * KCH + kc) * 128
                            nc.tensor.matmul(ps[:],
                                             lhsT=whi_sb[:, w0:w0 + 128],
                                             rhs=xt_sb[:, kc, :],
                                             start=(kc == 0), stop=(kc == KCH - 1))
                        xo = gpool.tile([128, MTW], F32)
                        nc.scalar.activation(out=xo[:], in_=ps[:], func=AF.Identity,
                                             bias=xbias_sb[:, n_:n_ + 1], scale=1.0)
                        nc.sync.dma_start(out=xg_w[:, n_, mt * MTW:(mt + 1) * MTW],
                                          in_=xo[:])

            # ---------------- Phase 2: the time recurrence ----------------
            with tc.tile_pool(name="p2s", bufs=2) as spool, \
                 tc.tile_pool(name="p2ps", bufs=2, space="PSUM") as ps2:
                with tc.For_i(0, S, 1, hint_engines=(PE,)) as iv:
                    xg_sb = spool.tile([128, NCH * BL], F32)
                    nc.gpsimd.dma_start(
                        out=xg_sb[:].rearrange("p (n o b) -> p n o b", n=NCH, o=1),
                        in_=xg_r[:, :, ds(iv, 1), :])

                    r_ps = ps2.tile([128, KCH * BL], F32)
                    z_ps = ps2.tile([128, KCH * BL], F32)
                    n_ps = ps2.tile([128, KCH * BL], F32)
                    for g, ps in enumerate((r_ps, z_ps, n_ps)):
                        for mc in range(KCH):
                            for kc in range(KCH):
                                w0 = (((g * KCH + mc) * KCH) + kc) * 128
                                nc.tensor.matmul(ps[:, mc * BL:(mc + 1) * BL],
                                                 lhsT=whh_sb[:, w0:w0 + 128],
                                                 rhs=hb[:, kc * BL:(kc + 1) * BL],
                                                 start=(kc == 0),
                                                 stop=(kc == KCH - 1))

                    ar = spool.tile([128, KCH * BL], F32)
                    az = spool.tile([128, KCH * BL], F32)
                    rt = spool.tile([128, KCH * BL], F32)
                    zt = spool.tile([128, KCH * BL], F32)
                    k1 = spool.tile([128, KCH * BL], F32)
                    an = spool.tile([128, KCH * BL], F32)
                    nt = spool.tile([128, KCH * BL], F32)
                    e = spool.tile([128, KCH * BL], F32)
                    f = spool.tile([128, KCH * BL], F32)

                    nc.vector.tensor_add(ar[:], r_ps[:], xg_sb[:, 0:64])
                    nc.scalar.activation(out=rt[:], in_=ar[:], func=AF.Sigmoid)
                    nc.vector.tensor_add(az[:], z_ps[:], xg_sb[:, 64:128])
                    nc.scalar.activation(out=zt[:], in_=az[:], func=AF.Sigmoid)
                    nc.vector.tensor_add(k1[:], n_ps[:], bhn_sb[:])
                    nc.vector.tensor_mul(k1[:], rt[:], k1[:])
                    nc.vector.tensor_add(an[:], k1[:], xg_sb[:, 128:192])
                    nc.scalar.activation(out=nt[:], in_=an[:], func=AF.Tanh)
                    # h = n + z*(h - n)
                    nc.vector.tensor_sub(e[:], hf[:], nt[:])
                    nc.vector.tensor_mul(f[:], zt[:], e[:])
                    nc.vector.tensor_add(hf[:], nt[:], f[:])
                    nc.vector.tensor_copy(hb[:], hf[:])
                    nc.gpsimd.dma_start(
                        out=outT_v[:, ds(iv, 1), :, :],
                        in_=hf[:].rearrange("p (o hc b) -> p o hc b", o=1, hc=KCH))

    nc.compile()
    return nc


def _prep_weights(w_ir, w_iz, w_in, w_hr, w_hz, w_hn,
                  b_ir, b_iz, b_in, b_hr, b_hz, b_hn):
    Wi = np.concatenate([w_ir, w_iz, w_in], axis=0)  # [3H, I]
    Wh = np.concatenate([w_hr, w_hz, w_hn], axis=0)  # [3H, H]
    # whX[p, ((n*KCH+kc)*128 + m)] = W[n*128+m, kc*128+p]
    whi = np.ascontiguousarray(
        Wi.reshape(NCH, 128, KCH, 128).transpose(3, 0, 2, 1).reshape(128, -1)
    ).astype(ml_dtypes.bfloat16)
    whh = np.ascontiguousarray(
        Wh.reshape(NCH, 128, KCH, 128).transpose(3, 0, 2, 1).reshape(128, -1)
    ).astype(ml_dtypes.bfloat16)
    cb = np.concatenate([b_ir + b_hr, b_iz + b_hz, b_in])  # [3H]
    xbias = np.ascontiguousarray(cb.reshape(NCH, 128).T).astype(np.float32)
    bhn_t = b_hn.reshape(KCH, 128).T  # [128, 8]
    bhn = np.ascontiguousarray(
        np.repeat(bhn_t[:, :, None], BL, axis=2).reshape(128, KCH * BL)
    ).astype(np.float32)
    return whi, whh, xbias, bhn


def kernel(inputs, hidden_states, w_ir, w_iz, w_in, b_ir, b_iz, b_in,
           w_hr, w_hz, w_hn, b_hr, b_hz, b_hn):
    global _built
    if _built is None:
        _built = _build()
    nc = _built

    whi, whh, xbias, bhn = _prep_weights(
        np.asarray(w_ir), np.asarray(w_iz), np.asarray(w_in),
        np.asarray(w_hr), np.asarray(w_hz), np.asarray(w_hn),
        np.asarray(b_ir), np.asarray(b_iz), np.asarray(b_in),
        np.asarray(b_hr), np.asarray(b_hz), np.asarray(b_hn))

    inputs = np.asarray(inputs, dtype=np.float32)
    h0_full = np.asarray(hidden_states, dtype=np.float32)[0]  # [B, H]

    in_maps = []
    for c in range(NCORES):
        xl = inputs[c * BL:(c + 1) * BL]          # [8, S, I]
        xtl = np.ascontiguousarray(
            xl.transpose(2, 1, 0).reshape(I, S * BL)).astype(ml_dtypes.bfloat16)
        hl = h0_full[c * BL:(c + 1) * BL]         # [8, H]
        h0p = np.ascontiguousarray(
            hl.reshape(BL, KCH, 128).transpose(2, 1, 0).reshape(128, KCH * BL)
        ).astype(np.float32)
        in_maps.append({"xt": xtl, "whi": whi, "whh": whh,
                        "xbias": xbias, "bhn": bhn, "h0": h0p})

    res = run_bass_kernel_spmd(nc, in_maps, core_ids=list(range(NCORES)),
                               trace=False)

    outputs = np.empty((B, S, H), dtype=np.float32)
    for c in range(NCORES):
        ot = res.results[c]["outT"]               # [S, H, BL]
        outputs[c * BL:(c + 1) * BL] = ot.transpose(2, 0, 1)
    h_final = outputs[:, -1, :][None]             # [1, B, H]
    return outputs, h_final
